# revision 1
# baseline (speedup 1.0000x reference)
"""DynamicSincConv1d Trainium2 kernel.

Data-parallel over batch: 8 batch elements -> 8 NeuronCores, one full
pipeline per core. All heavy math (conv, sinc synthesis, r/irFFTs,
complex filtering, overlap-add) runs on-device; the host only reshapes
inputs into DMA-friendly layouts and reassembles the output.

Math notes:
 - STFT/irFFT are matmuls against host-baked DFT matrices with the
   hann/blackman windows folded in.
 - The sinc bank is symmetric about k=128 (fir window is symmetric and
   sinc is even up to the 1e-6 eps shift), so only d = k-128 in [0,128)
   is materialized; fir[0] = 0 kills the k=0 column exactly.  This also
   makes every filter spectrum real: fft_filt[fb] = (-1)^fb * R[fb],
   which turns the complex multiply into a real scaling of X.
"""

import math
import numpy as np

B, CI, I, O, S = 8, 32, 2, 4, 4
K, HOP, T = 256, 64, 65536
F = T // HOP + 1            # 1025 frames
H = (T + K) // HOP          # 1028 hops in the padded signal
FPAD = H                    # fr frame slots: [zero][f=0..1024][zero][spare]
EPS = 1e-6
PI = math.pi
FTILES = [(0, 512), (512, 512), (1024, 1)]

_prog_cache = {}


def _consts():
    n = np.arange(K, dtype=np.float64)
    ola = 0.5 * (1.0 - np.cos(2.0 * np.pi * n / K))
    fir = 0.42 - 0.5 * np.cos(2.0 * np.pi * n / K) + 0.08 * np.cos(4.0 * np.pi * n / K)

    d = np.arange(128, dtype=np.float64)
    ta_col = (PI * d / K).astype(np.float32).reshape(128, 1)

    # M1[d, fb] = (-1)^fb * c_d * fir[128+d]/(S*K) * cos(2*pi*d*fb/K)
    fb = np.arange(K // 2 + 1, dtype=np.float64)     # 0..128
    cd = np.where(d == 0, 1.0, 2.0)
    m1 = (((-1.0) ** fb)[None, :] * cd[:, None] * fir[128 + d.astype(int)][:, None]
          / (S * K) * np.cos(2.0 * np.pi * np.outer(d, fb) / K))
    m1 = m1.astype(np.float32)                        # [128, 129]
    m1a = np.ascontiguousarray(m1[:, 0:128])          # cols fb 0..127
    m1b = np.concatenate([m1[:, 128:129], m1[:, 1:128]], axis=1)  # [fb128, fb1..127]

    # STFT weights: wx[j][r, col] ; k = 64 j + r ; fbpack col layout
    kk = np.arange(K, dtype=np.float64)
    ang = 2.0 * np.pi * np.outer(kk, fb) / K          # [256, 129]
    wre = ola[:, None] * np.cos(ang)                  # [256, 129]
    wim = -ola[:, None] * np.sin(ang)
    colsA = wre[:, 0:128]                             # [256, 128]
    colsB = np.concatenate([wre[:, 128:129], wim[:, 1:128]], axis=1)
    wx_full = np.concatenate([colsA, colsB], axis=1)  # [256, 256]
    wx = wx_full.reshape(4, 64, 256)                  # [j, r, 256]
    wx_h = np.ascontiguousarray(wx.transpose(1, 0, 2).reshape(64, 1024)).astype(np.float32)

    # iSTFT: IC[fbpack_row, n] with ola folded
    nn_ = np.arange(K, dtype=np.float64)
    cp = np.where(fb == 0, 1.0, 2.0)
    icre = (cp[:, None] / K) * np.cos(2.0 * np.pi * np.outer(fb, nn_) / K) * ola[None, :]   # [129, 256]
    icim = (-2.0 / K) * np.sin(2.0 * np.pi * np.outer(fb, nn_) / K) * ola[None, :]          # [129, 256]
    ica = np.ascontiguousarray(icre[0:128]).astype(np.float32)                               # [128, 256]
    icb = np.concatenate([icre[128:129], icim[1:128]], axis=0).astype(np.float32)            # [128, 256]

    # env / inverse, arranged [r, p]
    ola2 = ola * ola
    env_q = np.zeros((H, 64), dtype=np.float64)
    for j in range(4):
        env_q[j:F + j, :] += ola2[64 * j:64 * j + 64][None, :]
    invt = (1.0 / env_q[2:2 + 1024, :]).T.astype(np.float32)  # [64, 1024]
    invt = np.ascontiguousarray(invt)

    e32 = np.zeros((32, 32, 128), dtype=np.float32)
    for q in range(32):
        e32[q, q, :] = 1.0
    e32 = np.ascontiguousarray(e32.transpose(0, 1, 2).reshape(32, 4096, order='F') if False else e32.swapaxes(0, 0).reshape(32, 32*128))
    # e32[k, q*128 + d] = 1 iff k == q   (built as [k? ...] fix below)
    e32 = np.zeros((32, 32 * 128), dtype=np.float32)
    for q in range(32):
        e32[q, q * 128:(q + 1) * 128] = 1.0
    return dict(ola=ola.astype(np.float32), ta_col=ta_col, m1a=m1a, m1b=m1b,
                wx_h=wx_h, ica=ica, icb=icb, invt=invt, e32=e32)


def _build_program():
    import concourse.bacc as bacc
    import concourse.mybir as mybir
    import concourse.tile as tile

    dt = mybir.dt.float32
    AF = mybir.ActivationFunctionType

    nc = bacc.Bacc("TRN2", target_bir_lowering=False, debug=False, num_devices=8)

    d_in = nc.dram_tensor("d_in", [16, 128, H], dt, kind="ExternalInput")
    xd_in = nc.dram_tensor("xd_in", [64, 2 * H], dt, kind="ExternalInput")
    w1t_in = nc.dram_tensor("w1t_in", [128, 2048], dt, kind="ExternalInput")
    w2t_in = nc.dram_tensor("w2t_in", [32, 64], dt, kind="ExternalInput")
    b1_in = nc.dram_tensor("b1_in", [32, 1], dt, kind="ExternalInput")
    b2a_in = nc.dram_tensor("b2a_in", [32, 1], dt, kind="ExternalInput")
    b2w_in = nc.dram_tensor("b2w_in", [32, 1], dt, kind="ExternalInput")
    ta_in = nc.dram_tensor("ta_in", [128, 1], dt, kind="ExternalInput")
    m1a_in = nc.dram_tensor("m1a_in", [128, 128], dt, kind="ExternalInput")
    m1b_in = nc.dram_tensor("m1b_in", [128, 128], dt, kind="ExternalInput")
    wx_in = nc.dram_tensor("wx_in", [64, 1024], dt, kind="ExternalInput")
    ica_in = nc.dram_tensor("ica_in", [128, 256], dt, kind="ExternalInput")
    icb_in = nc.dram_tensor("icb_in", [128, 256], dt, kind="ExternalInput")
    invt_in = nc.dram_tensor("invt_in", [64, 1024], dt, kind="ExternalInput")
    bias_in = nc.dram_tensor("bias_in", [64, 4], dt, kind="ExternalInput")
    e32_in = nc.dram_tensor("e32_in", [32, 4096], dt, kind="ExternalInput")
    yt_out = nc.dram_tensor("yt_out", [64, 4096], dt, kind="ExternalOutput")

    PIEPS = PI * EPS

    with tile.TileContext(nc) as tc:
        with tc.tile_pool(name="cpool", bufs=1) as cpool:
            w1t_sb = cpool.tile([128, 2048], dt, tag="w1t")
            w2t_sb = cpool.tile([32, 64], dt, tag="w2t")
            b1_sb = cpool.tile([32, 1], dt, tag="b1")
            b2a_sb = cpool.tile([32, 1], dt, tag="b2a")
            b2w_sb = cpool.tile([32, 1], dt, tag="b2w")
            ta_sb = cpool.tile([128, 1], dt, tag="ta")
            m1a_sb = cpool.tile([128, 128], dt, tag="m1a")
            m1b_sb = cpool.tile([128, 128], dt, tag="m1b")
            wx_sb = cpool.tile([64, 1024], dt, tag="wx")
            ica_sb = cpool.tile([128, 256], dt, tag="ica")
            icb_sb = cpool.tile([128, 256], dt, tag="icb")
            invt_sb = cpool.tile([64, 1024], dt, tag="invt")
            bias_sb = cpool.tile([64, 4], dt, tag="bias")
            e32_sb = cpool.tile([32, 4096], dt, tag="e32")
            xd_sb = cpool.tile([64, 2 * H], dt, tag="xd")
            h_sb = cpool.tile([32, F], dt, tag="h")
            amp_sb = cpool.tile([32, F], dt, tag="amp")
            wid_sb = cpool.tile([32, F], dt, tag="wid")
            yt_sb = cpool.tile([64, 4096], dt, tag="yt")
            xa_sb = cpool.tile([128, 2 * F], dt, tag="xa")
            xb_sb = cpool.tile([128, 2 * F], dt, tag="xb")

            for t_sb, t_in in ((w1t_sb, w1t_in), (w2t_sb, w2t_in), (b1_sb, b1_in),
                               (b2a_sb, b2a_in), (b2w_sb, b2w_in),
                               (ta_sb, ta_in), (m1a_sb, m1a_in),
                               (m1b_sb, m1b_in), (wx_sb, wx_in), (ica_sb, ica_in),
                               (icb_sb, icb_in), (invt_sb, invt_in),
                               (bias_sb, bias_in), (xd_sb, xd_in),
                               (e32_sb, e32_in)):
                nc.sync.dma_start(t_sb[:], t_in[:])

            # ---- stage 1: conditioning conv -> h [32, F] ----
            with tc.tile_pool(name="dpool", bufs=1) as dpool, \
                 tc.tile_pool(name="lp", bufs=2) as lpool, \
                 tc.tile_pool(name="ps1", bufs=2, space="PSUM") as ps1:
                dts = []
                for c in range(16):
                    dtile = dpool.tile([128, H], dt, tag=f"d{c}")
                    nc.sync.dma_start(dtile[:], d_in[c])
                    dts.append(dtile)
                for (f0, nf) in FTILES:
                    ps = ps1.tile([32, nf], dt, tag="ps1")
                    kmax = 16 * 4 - 1
                    idx = 0
                    for c in range(16):
                        for j in range(4):
                            nc.tensor.matmul(
                                ps[:],
                                w1t_sb[:, (c * 4 + j) * 32:(c * 4 + j + 1) * 32],
                                dts[c][:, f0 + j:f0 + j + nf],
                                start=(idx == 0), stop=(idx == kmax))
                            idx += 1
                    nc.scalar.activation(h_sb[:, f0:f0 + nf], ps[:], AF.Identity,
                                         bias=b1_sb[:, 0:1])
                    lt = lpool.tile([32, nf], dt, tag="lt")
                    nc.vector.tensor_scalar(lt[:], h_sb[:, f0:f0 + nf], 0.01, None,
                                            mybir.AluOpType.mult)
                    nc.vector.tensor_max(h_sb[:, f0:f0 + nf], h_sb[:, f0:f0 + nf], lt[:])

            # ---- stage 2: 1x1 conv + tanh -> amp/width [32, F] each ----
            with tc.tile_pool(name="ps2", bufs=2, space="PSUM") as ps2:
                for (f0, nf) in FTILES:
                    pa = ps2.tile([32, nf], dt, tag="ps2a")
                    nc.tensor.matmul(pa[:], w2t_sb[:, 0:32], h_sb[:, f0:f0 + nf],
                                     start=True, stop=True)
                    nc.scalar.activation(amp_sb[:, f0:f0 + nf], pa[:], AF.Tanh,
                                         bias=b2a_sb[:, 0:1])
                    pw = ps2.tile([32, nf], dt, tag="ps2w")
                    nc.tensor.matmul(pw[:], w2t_sb[:, 32:64], h_sb[:, f0:f0 + nf],
                                     start=True, stop=True)
                    nc.scalar.activation(wid_sb[:, f0:f0 + nf], pw[:], AF.Tanh,
                                         bias=b2w_sb[:, 0:1])

            # ---- stage 4 (early): STFT of x -> XA/XB [128, (i,f)] ----
            with tc.tile_pool(name="ps4", bufs=2, space="PSUM") as ps4:
                for i in range(2):
                    for (mt, xdst) in ((0, xa_sb), (1, xb_sb)):
                        for (f0, nf) in FTILES:
                            ps = ps4.tile([128, nf], dt, tag="ps4")
                            for j in range(4):
                                nc.tensor.matmul(
                                    ps[:],
                                    wx_sb[:, j * 256 + mt * 128: j * 256 + mt * 128 + 128],
                                    xd_sb[:, i * H + f0 + j:i * H + f0 + j + nf],
                                    start=(j == 0), stop=(j == 3))
                            nc.scalar.activation(xdst[:, i * F + f0:i * F + f0 + nf],
                                                 ps[:], AF.Copy)

            # ---- per o-half: sinc synth + DFT + cmul + iSTFT + OLA ----
            for half in range(2):
                with tc.tile_pool(name="fp", bufs=1) as fppool:
                    fpa = fppool.tile([128, 4 * F], dt, tag="fpa")
                    fpb = fppool.tile([128, 4 * F], dt, tag="fpb")
                    with tc.tile_pool(name="s3w", bufs=2) as wpool, \
                         tc.tile_pool(name="s3f", bufs=5) as fpool, \
                         tc.tile_pool(name="bps", bufs=1, space="PSUM") as bps, \
                         tc.tile_pool(name="ps3", bufs=2, space="PSUM") as ps3:
                        for oil in range(4):          # oi local within half
                            oi = half * 4 + oil
                            fts = []
                            for s in range(4):
                                ois = oi * 4 + s
                                w_b = bps.tile([128, F], dt, tag="wb")
                                a_b = bps.tile([128, F], dt, tag="ab")
                                for (f0, nf) in FTILES:
                                    nc.tensor.matmul(
                                        w_b[:, f0:f0 + nf],
                                        e32_sb[:, ois * 128:(ois + 1) * 128],
                                        wid_sb[:, f0:f0 + nf],
                                        start=True, stop=True)
                                    nc.tensor.matmul(
                                        a_b[:, f0:f0 + nf],
                                        e32_sb[:, ois * 128:(ois + 1) * 128],
                                        amp_sb[:, f0:f0 + nf],
                                        start=True, stop=True)
                                z = wpool.tile([128, F], dt, tag="z")
                                nc.vector.tensor_scalar(
                                    z[:], w_b[:], ta_sb[:, 0:1], PIEPS,
                                    mybir.AluOpType.mult, mybir.AluOpType.add)
                                sn = wpool.tile([128, F], dt, tag="sn")
                                nc.scalar.activation(sn[:], z[:], AF.Sin)
                                rv = wpool.tile([128, F], dt, tag="rv")
                                nc.vector.reciprocal_approx_fast(rv[:], z[:])
                                sr = wpool.tile([128, F], dt, tag="sr")
                                nc.vector.tensor_mul(sr[:], sn[:], rv[:])
                                ft = fpool.tile([128, F], dt, tag="ft")
                                nc.vector.tensor_mul(ft[:], sr[:], a_b[:])
                                fts.append(ft)
                            for (mt, m1sb, dest) in ((0, m1a_sb, fpa), (1, m1b_sb, fpb)):
                                for (f0, nf) in FTILES:
                                    ps = ps3.tile([128, nf], dt, tag="ps3")
                                    for s in range(4):
                                        nc.tensor.matmul(ps[:], m1sb[:],
                                                         fts[s][:, f0:f0 + nf],
                                                         start=(s == 0), stop=(s == 3))
                                    nc.scalar.activation(
                                        dest[:, oil * F + f0:oil * F + f0 + nf],
                                        ps[:], AF.Copy)

                    # ---- stage 5: Y = X * F' (F' real), sum over i ----
                    with tc.tile_pool(name="yp", bufs=1) as ypool:
                        ya = ypool.tile([128, 2 * F], dt, tag="ya")
                        yb = ypool.tile([128, 2 * F], dt, tag="yb")
                        with tc.tile_pool(name="ct", bufs=2) as ctpool:
                            for ol in range(2):
                                o_l2 = ol * 2
                                ta_t = ctpool.tile([128, 2 * F], dt, tag="cta")
                                nc.vector.tensor_mul(
                                    ta_t[:], xa_sb[:, 0:2 * F],
                                    fpa[:, o_l2 * F:(o_l2 + 2) * F])
                                nc.vector.tensor_add(ya[:, ol * F:(ol + 1) * F],
                                                     ta_t[:, 0:F], ta_t[:, F:2 * F])
                                tb_t = ctpool.tile([128, 2 * F], dt, tag="ctb")
                                nc.gpsimd.tensor_mul(
                                    tb_t[:], xb_sb[:, 0:2 * F],
                                    fpb[:, o_l2 * F:(o_l2 + 2) * F])
                                nc.gpsimd.tensor_add(yb[:, ol * F:(ol + 1) * F],
                                                     tb_t[:, 0:F], tb_t[:, F:2 * F])

                        # ---- stage 6: iSTFT + OLA for this half's 2 o's ----
                        with tc.tile_pool(name="frp", bufs=1) as frpool, \
                             tc.tile_pool(name="ps6", bufs=2, space="PSUM") as ps6, \
                             tc.tile_pool(name="olat", bufs=2) as olat:
                            frs = []
                            for j in range(4):
                                frj = frpool.tile([64, 2 * FPAD], dt, tag=f"fr{j}")
                                for ol in range(2):
                                    nc.gpsimd.memset(frj[:, ol * FPAD:ol * FPAD + 1], 0.0)
                                    nc.gpsimd.memset(
                                        frj[:, ol * FPAD + 1026:ol * FPAD + 1027], 0.0)
                                frs.append(frj)
                            for j in range(4):
                                for ol in range(2):
                                    ps = ps6.tile([64, F], dt, tag="ps6")
                                    for (f0, nf) in FTILES:
                                        nc.tensor.matmul(
                                            ps[:, f0:f0 + nf],
                                            ica_sb[:, j * 64:(j + 1) * 64],
                                            ya[:, ol * F + f0:ol * F + f0 + nf],
                                            start=True, stop=False)
                                        nc.tensor.matmul(
                                            ps[:, f0:f0 + nf],
                                            icb_sb[:, j * 64:(j + 1) * 64],
                                            yb[:, ol * F + f0:ol * F + f0 + nf],
                                            start=False, stop=True)
                                    nc.scalar.activation(
                                        frs[j][:, ol * FPAD + 1:ol * FPAD + 1 + F],
                                        ps[:], AF.Copy)
                            for ol in range(2):
                                o = half * 2 + ol
                                eng = nc.vector if ol == 0 else nc.gpsimd
                                t1 = olat.tile([64, 1024], dt, tag="t1")
                                t2 = olat.tile([64, 1024], dt, tag="t2")
                                eng.tensor_add(t1[:],
                                               frs[0][:, ol * FPAD + 3:ol * FPAD + 3 + 1024],
                                               frs[1][:, ol * FPAD + 2:ol * FPAD + 2 + 1024])
                                eng.tensor_add(t2[:],
                                               frs[2][:, ol * FPAD + 1:ol * FPAD + 1 + 1024],
                                               frs[3][:, ol * FPAD + 0:ol * FPAD + 0 + 1024])
                                eng.tensor_add(t1[:], t1[:], t2[:])
                                eng.tensor_mul(t1[:], t1[:], invt_sb[:])
                                eng.tensor_scalar(
                                    yt_sb[:, o * 1024:(o + 1) * 1024], t1[:],
                                    bias_sb[:, o:o + 1], None, mybir.AluOpType.add)

            nc.sync.dma_start(yt_out[:], yt_sb[:])

    nc.compile()
    return nc


def _prep_inputs(x, conditioning, w1, b1, w2, b2, bias):
    c = _consts()
    x = np.asarray(x, dtype=np.float32)
    conditioning = np.asarray(conditioning, dtype=np.float32)
    w1 = np.asarray(w1, dtype=np.float32)
    b1 = np.asarray(b1, dtype=np.float32)
    w2 = np.asarray(w2, dtype=np.float32)
    b2 = np.asarray(b2, dtype=np.float32)
    bias = np.asarray(bias, dtype=np.float32)

    w1t = w1.reshape(32, 32, 4, 64).transpose(1, 3, 2, 0).reshape(2048, 4, 32)
    w1t_sb = np.ascontiguousarray(
        w1t.reshape(16, 128, 4, 32).transpose(1, 0, 2, 3).reshape(128, 2048))
    w2t = np.ascontiguousarray(w2[:, :, 0].T)            # [32, 64]
    bias64 = np.tile(bias.reshape(1, 4), (64, 1)).astype(np.float32)

    shared = {
        "w1t_in": w1t_sb, "w2t_in": w2t,
        "b1_in": b1.reshape(32, 1).copy(),
        "b2a_in": b2[:32].reshape(32, 1).copy(),
        "b2w_in": b2[32:].reshape(32, 1).copy(),
        "ta_in": c["ta_col"], "m1a_in": c["m1a"], "m1b_in": c["m1b"],
        "wx_in": c["wx_h"], "ica_in": c["ica"], "icb_in": c["icb"],
        "invt_in": c["invt"], "bias_in": bias64, "e32_in": c["e32"],
    }
    in_maps = []
    for b in range(B):
        condpad = np.zeros((CI, T + K), dtype=np.float32)
        condpad[:, 128:128 + T] = conditioning[b]
        d = condpad.reshape(CI, H, 64).transpose(0, 2, 1).reshape(2048, H)
        d = np.ascontiguousarray(d.reshape(16, 128, H))
        xp = np.pad(x[b], ((0, 0), (128, 128)), mode="reflect")
        xd = np.ascontiguousarray(
            xp.reshape(2, H, 64).transpose(0, 2, 1).reshape(2, 64, H)
            .transpose(1, 0, 2).reshape(64, 2 * H))
        m = dict(shared)
        m["d_in"] = d
        m["xd_in"] = xd
        in_maps.append(m)
    return in_maps


def _assemble(results):
    y = np.empty((B, O, T), dtype=np.float32)
    for b in range(B):
        yt = results[b]["yt_out"]                        # [64, 4096]
        y[b] = yt.reshape(64, 4, 1024).transpose(1, 2, 0).reshape(4, T)
    return y


def kernel(x, conditioning, w1, b1, w2, b2, bias):
    from concourse.bass_utils import run_bass_kernel_spmd
    if "nc" not in _prog_cache:
        _prog_cache["nc"] = _build_program()
    nc = _prog_cache["nc"]
    in_maps = _prep_inputs(x, conditioning, w1, b1, w2, b2, bias)
    res = run_bass_kernel_spmd(nc, in_maps, core_ids=list(range(B)))
    return _assemble(res.results)



# revision 13
# speedup vs baseline: 5.5905x; 5.5905x over previous
"""DynamicSincConv1d Trainium2 kernel (v2).

Data-parallel over batch: 8 batch elements -> 8 NeuronCores. All heavy
math runs on-device; the host reshapes inputs into DMA-friendly layouts
and reassembles the output.

Key algorithmic moves vs a direct translation:
 - The windowed-sinc bank is symmetric about k=128, so every filter
   spectrum is real: the complex multiply becomes a real scaling of X
   (fbpack layout: pack a = Re fb 0..127, pack b = [fb128, Im fb 1..127]).
 - sinc(w*t) is approximated by a degree-3 polynomial in (w*t)^2
   (max err 2e-6 over the reachable |w|<=1, |t|<=0.5 range).  The
   filter synthesis + rFFT then factorizes through 4 moments
   P_j = amp*w^(2j): R = WS @ Pm with host-baked WS.  This removes the
   per-(o,i,s) sinc/sin/reciprocal elementwise work and two matmul
   layers entirely.
 - Stage-1 conv uses m=128 output tiles (4 kernel-shifts x 32 channels)
   and a cheap cross-partition j-sum, 4x fewer matmul rows.
 - iSTFT overlap-add folds into PSUM accumulation with shifted rhs
   slices; the window-square normalization is periodic except at the
   two boundary hops, so it reduces to a per-partition tensor_scalar.
 - Matmuls run in fp16/fp32r (1 cycle/row vs 4 for fp32); elementwise
   stages run in fp16 (2x DVE throughput).
"""

import math
import numpy as np

B, CI, I, O, S = 8, 32, 2, 4, 4
K, HOP, T = 256, 64, 65536
F = T // HOP + 1            # 1025 frames
H = (T + K) // HOP          # 1028
PI = math.pi
FT_F = [(0, 512), (512, 512), (1024, 1)]     # tiles of F
FT_H = [(0, 512), (512, 512), (1024, 4)]     # tiles of H

_prog_cache = {}


def _consts():
    n = np.arange(K, dtype=np.float64)
    ola = 0.5 * (1.0 - np.cos(2.0 * PI * n / K))
    fir = 0.42 - 0.5 * np.cos(2.0 * PI * n / K) + 0.08 * np.cos(4.0 * PI * n / K)

    d = np.arange(128, dtype=np.float64)
    fb = np.arange(K // 2 + 1, dtype=np.float64)
    cd = np.where(d == 0, 1.0, 2.0)
    m1full = (((-1.0) ** fb)[None, :] * cd[:, None] * fir[128 + d.astype(int)][:, None]
              / (S * K) * np.cos(2.0 * PI * np.outer(d, fb) / K))     # [128, 129]

    # sinc_n(u) ~= sum_j c_j u^(2j) on u in [0, 0.503]
    us = np.linspace(0, 0.503, 6001)
    V = np.vander(us * us, 4, increasing=True)
    c, *_ = np.linalg.lstsq(V, np.sinc(us), rcond=None)

    pw = (d[:, None] / K) ** (2 * np.arange(4)[None, :])              # [128, 4]
    M2full = np.einsum("df,dj->fj", m1full, pw) * c[None, :]          # [129, 4]
    M2a = M2full[0:128]
    M2b = np.concatenate([M2full[128:129], M2full[1:128]], axis=0)    # [128, 4]

    WS = np.zeros((128, 16, 128), dtype=np.float64)
    for oi in range(8):
        for j in range(4):
            for s in range(4):
                WS[32 * j + 4 * oi + s, oi * 2 + 0, :] = M2a[:, j]
                WS[32 * j + 4 * oi + s, oi * 2 + 1, :] = M2b[:, j]
    WS = np.ascontiguousarray(WS.reshape(128, 2048)).astype(np.float16)

    kk = np.arange(K, dtype=np.float64)
    ang = 2.0 * PI * np.outer(kk, fb) / K
    wre = ola[:, None] * np.cos(ang)
    wim = -ola[:, None] * np.sin(ang)
    colsA = wre[:, 0:128]
    colsB = np.concatenate([wre[:, 128:129], wim[:, 1:128]], axis=1)
    wx_full = np.concatenate([colsA, colsB], axis=1).reshape(4, 64, 256)
    wx_h = np.ascontiguousarray(
        wx_full.transpose(1, 0, 2).reshape(64, 1024)).astype(np.float16)

    cp = np.where(fb == 0, 1.0, 2.0)
    icre = (cp[:, None] / K) * np.cos(2.0 * PI * np.outer(fb, n) / K) * ola[None, :]
    icim = (-2.0 / K) * np.sin(2.0 * PI * np.outer(fb, n) / K) * ola[None, :]
    ica = np.ascontiguousarray(icre[0:128]).astype(np.float16)        # [128, 256]
    icb = np.concatenate([icre[128:129], icim[1:128]], axis=0).astype(np.float16)

    ola2 = ola * ola
    r = np.arange(64)
    env_int = sum(ola2[64 * j + r] for j in range(4))
    env_p0 = sum(ola2[64 * j + r] for j in (0, 1, 2))
    env_p1023 = sum(ola2[64 * j + r] for j in (1, 2, 3))
    inv3 = np.stack([1.0 / env_int, 1.0 / env_p0, 1.0 / env_p1023],
                    axis=1).astype(np.float32)                        # [64, 3]

    return dict(WS=WS, wx_h=wx_h, ica=ica, icb=icb, inv3=inv3)


def _build_program():
    import concourse.bacc as bacc
    import concourse.mybir as mybir
    import concourse.tile as tile

    f32 = mybir.dt.float32
    f16 = mybir.dt.float16
    f32r = mybir.dt.float32r
    AF = mybir.ActivationFunctionType
    ALU = mybir.AluOpType

    nc = bacc.Bacc("TRN2", target_bir_lowering=False, debug=False, num_devices=8)

    eye_in = nc.dram_tensor("eye_in", [128, 128], f16, kind="ExternalInput")
    d_in = nc.dram_tensor("d_in", [16, 128, H], f16, kind="ExternalInput")
    xd_in = nc.dram_tensor("xd_in", [64, 2 * H], f16, kind="ExternalInput")
    w1n_in = nc.dram_tensor("w1n_in", [128, 2048], f16, kind="ExternalInput")
    w2t_in = nc.dram_tensor("w2t_in", [32, 64], f16, kind="ExternalInput")
    ws_in = nc.dram_tensor("ws_in", [128, 2048], f16, kind="ExternalInput")
    wx_in = nc.dram_tensor("wx_in", [64, 1024], f16, kind="ExternalInput")
    ica_in = nc.dram_tensor("ica_in", [128, 256], f16, kind="ExternalInput")
    icb_in = nc.dram_tensor("icb_in", [128, 256], f16, kind="ExternalInput")
    b1_in = nc.dram_tensor("b1_in", [32, 1], f32, kind="ExternalInput")
    b2a_in = nc.dram_tensor("b2a_in", [32, 1], f32, kind="ExternalInput")
    b2w_in = nc.dram_tensor("b2w_in", [32, 1], f32, kind="ExternalInput")
    inv3_in = nc.dram_tensor("inv3_in", [64, 3], f32, kind="ExternalInput")
    bias_in = nc.dram_tensor("bias_in", [64, 4], f32, kind="ExternalInput")
    yt_out = nc.dram_tensor("yt_out", [64, 4096], f32, kind="ExternalOutput")

    with tile.TileContext(nc) as tc:
        with tc.tile_pool(name="cpool", bufs=1) as cpool:
            w1n_sb = cpool.tile([128, 2048], f16, tag="w1n")
            w2t_sb = cpool.tile([32, 64], f16, tag="w2t")
            ws_sb = cpool.tile([128, 2048], f16, tag="ws")
            wx_sb = cpool.tile([64, 1024], f16, tag="wx")
            ica_sb = cpool.tile([128, 256], f16, tag="ica")
            icb_sb = cpool.tile([128, 256], f16, tag="icb")
            b1_sb = cpool.tile([32, 1], f32, tag="b1")
            b2a_sb = cpool.tile([32, 1], f32, tag="b2a")
            b2w_sb = cpool.tile([32, 1], f32, tag="b2w")
            inv3_sb = cpool.tile([64, 3], f32, tag="inv3")
            bias_sb = cpool.tile([64, 4], f32, tag="bias")
            xd_sb = cpool.tile([64, 2 * H], f16, tag="xd")
            xa_sb = cpool.tile([128, 2 * F], f16, tag="xa")
            xb_sb = cpool.tile([128, 2 * F], f16, tag="xb")
            eye_sb = cpool.tile([128, 128], f16, tag="eye")
            wid_sb = cpool.tile([32, F], f16, tag="wid")
            wsq_sb = cpool.tile([32, F], f16, tag="wsq")
            w4_sb = cpool.tile([32, F], f16, tag="w4")
            pm_sb = cpool.tile([128, F], f16, tag="pm")
            p1_sb = cpool.tile([32, F], f16, tag="p1")
            p2_sb = cpool.tile([32, F], f16, tag="p2")
            p3_sb = cpool.tile([32, F], f16, tag="p3")
            outs_sb = cpool.tile([128, H], f16, tag="outs")
            hb_sb = cpool.tile([32, F], f32, tag="hb")
            lt_sb = cpool.tile([32, F], f32, tag="lt")
            h_sb = cpool.tile([32, F], f16, tag="h")
            fpa_sb = cpool.tile([128, 8 * F], f16, tag="fpa")
            fpb_sb = cpool.tile([128, 8 * F], f16, tag="fpb")
            yt_sb = cpool.tile([64, 4096], f32, tag="yt")

            for t_sb, t_in in ((wx_sb, wx_in), (xd_sb, xd_in), (w1n_sb, w1n_in),
                               (w2t_sb, w2t_in), (ws_sb, ws_in), (eye_sb, eye_in),
                               (ica_sb, ica_in), (icb_sb, icb_in),
                               (b1_sb, b1_in), (b2a_sb, b2a_in), (b2w_sb, b2w_in),
                               (inv3_sb, inv3_in), (bias_sb, bias_in)):
                nc.sync.dma_start(t_sb[:], t_in[:])

            with tc.tile_pool(name="dpool", bufs=1) as dpool:
                dts = []
                for c in range(16):
                    dtile = dpool.tile([128, H], f16, tag=f"d{c}")
                    nc.sync.dma_start(dtile[:], d_in[c])
                    dts.append(dtile)

                # ---- stage 4: STFT of x -> xa/xb [128, (i,f)] f16 ----
                with tc.tile_pool(name="ps4", bufs=2, space="PSUM") as ps4:
                    for i in range(2):
                        for (mt, xdst) in ((0, xa_sb), (1, xb_sb)):
                            ps = ps4.tile([128, F], f32, tag="ps4")
                            for (f0, nf) in FT_F:
                                for j in range(4):
                                    nc.tensor.matmul(
                                        ps[:, f0:f0 + nf],
                                        wx_sb[:, j * 256 + mt * 128:
                                              j * 256 + mt * 128 + 128],
                                        xd_sb[:, i * H + f0 + j:i * H + f0 + j + nf],
                                        start=(j == 0), stop=(j == 3))
                            nc.scalar.activation(xdst[:, i * F:(i + 1) * F],
                                                 ps[:], AF.Copy)

                # ---- stage 1: conditioning conv, m=128 (4j x 32ch) ----
                with tc.tile_pool(name="ps1", bufs=1, space="PSUM") as ps1:
                    out_ps = ps1.tile([128, H], f32, tag="out1")
                    for c in range(16):
                        for (f0, nf) in FT_H:
                            nc.tensor.matmul(
                                out_ps[:, f0:f0 + nf],
                                w1n_sb[:, c * 128:(c + 1) * 128],
                                dts[c][:, f0:f0 + nf],
                                start=(c == 0), stop=(c == 15))
                    nc.scalar.activation(outs_sb[:], out_ps[:], AF.Copy)

                # j-sum via 4 shifted accumulating matmuls (identity lhsT),
                # then bias + leaky_relu
                with tc.tile_pool(name="psh", bufs=1, space="PSUM") as psh:
                    h_ps = psh.tile([32, F], f32, tag="hps")
                    for (f0, nf) in FT_F:
                        for j in range(4):
                            nc.tensor.matmul(
                                h_ps[:, f0:f0 + nf],
                                eye_sb[:, j * 32:(j + 1) * 32],
                                outs_sb[:, f0 + j:f0 + j + nf],
                                start=(j == 0), stop=(j == 3))
                    nc.scalar.activation(hb_sb[:], h_ps[:], AF.Identity,
                                         bias=b1_sb[:, 0:1])
                    nc.gpsimd.tensor_scalar(lt_sb[:], hb_sb[:], 0.01,
                                            None, ALU.mult)
                    nc.vector.tensor_max(h_sb[:], hb_sb[:], lt_sb[:])

            # ---- stage 2: 1x1 conv + tanh -> amp (pm rows 0:32) / wid ----
            with tc.tile_pool(name="ps2", bufs=4, space="PSUM") as ps2:
                for (f0, nf) in FT_F:
                    pa = ps2.tile([32, nf], f32, tag="ps2a")
                    nc.tensor.matmul(pa[:], w2t_sb[:, 0:32], h_sb[:, f0:f0 + nf],
                                     start=True, stop=True)
                    nc.scalar.activation(pm_sb[0:32, f0:f0 + nf], pa[:], AF.Tanh,
                                         bias=b2a_sb[:, 0:1])
                    pw = ps2.tile([32, nf], f32, tag="ps2w")
                    nc.tensor.matmul(pw[:], w2t_sb[:, 32:64], h_sb[:, f0:f0 + nf],
                                     start=True, stop=True)
                    nc.scalar.activation(wid_sb[:, f0:f0 + nf], pw[:], AF.Tanh,
                                         bias=b2w_sb[:, 0:1])

            # ---- moments: pm rows = [amp, amp*w^2, amp*w^4, amp*w^6] ----
            # computed on partitions 0:32, placed into pm rows via SBUF DMAs
            nc.vector.tensor_mul(wsq_sb[:], wid_sb[:], wid_sb[:])
            nc.gpsimd.tensor_mul(p1_sb[:], pm_sb[0:32, :], wsq_sb[:])
            nc.vector.tensor_mul(w4_sb[:], wsq_sb[:], wsq_sb[:])
            nc.vector.tensor_mul(p2_sb[:], pm_sb[0:32, :], w4_sb[:])
            nc.gpsimd.tensor_mul(p3_sb[:], p1_sb[:], w4_sb[:])
            nc.sync.dma_start(pm_sb[32:64, :], p1_sb[:])
            nc.sync.dma_start(pm_sb[64:96, :], p2_sb[:])
            nc.sync.dma_start(pm_sb[96:128, :], p3_sb[:])

            # ---- stage 3: R = WS @ Pm per (oi, pack) -> fpa/fpb f16 ----
            with tc.tile_pool(name="ps3", bufs=2, space="PSUM") as ps3:
                for oi in range(8):
                    for (p, dest) in ((0, fpa_sb), (1, fpb_sb)):
                        ps = ps3.tile([128, F], f32, tag="ps3")
                        for (f0, nf) in FT_F:
                            nc.tensor.matmul(
                                ps[:, f0:f0 + nf],
                                ws_sb[:, (oi * 2 + p) * 128:(oi * 2 + p + 1) * 128],
                                pm_sb[:, f0:f0 + nf],
                                start=True, stop=True)
                        dst = dest[:, oi * F:(oi + 1) * F]
                        if p == 0:
                            nc.scalar.activation(dst, ps[:], AF.Copy)
                        else:
                            nc.vector.tensor_scalar(dst, ps[:], 1.0, None, ALU.mult)

            # ---- stage 5+6 per o: cmul, iSTFT with OLA in PSUM ----
            with tc.tile_pool(name="yp", bufs=2) as ypool, \
                 tc.tile_pool(name="ctp", bufs=2) as ctpool, \
                 tc.tile_pool(name="ps6", bufs=4, space="PSUM") as ps6:
                for o in range(4):
                    ya = ypool.tile([128, H], f16, tag="ya")
                    yb = ypool.tile([128, H], f16, tag="yb")
                    ta_t = ctpool.tile([128, 2 * F], f16, tag="cta")
                    tb_t = ctpool.tile([128, 2 * F], f16, tag="ctb")
                    o2 = 2 * o
                    nc.gpsimd.memset(ya[:, 0:1], 0.0)
                    nc.gpsimd.memset(ya[:, 1026:1028], 0.0)
                    nc.gpsimd.memset(yb[:, 0:1], 0.0)
                    nc.gpsimd.memset(yb[:, 1026:1028], 0.0)
                    nc.gpsimd.tensor_mul(ta_t[:], xa_sb[:],
                                         fpa_sb[:, o2 * F:(o2 + 2) * F])
                    nc.gpsimd.tensor_add(ya[:, 1:1 + F], ta_t[:, 0:F],
                                         ta_t[:, F:2 * F])
                    nc.gpsimd.tensor_mul(tb_t[:], xb_sb[:],
                                         fpb_sb[:, o2 * F:(o2 + 2) * F])
                    nc.gpsimd.tensor_add(yb[:, 1:1 + F], tb_t[:, 0:F],
                                         tb_t[:, F:2 * F])

                    for pt in range(2):
                        ps = ps6.tile([64, 512], f32, tag="ps6")
                        idx = 0
                        for j in range(4):
                            c0 = pt * 512 + 3 - j
                            nc.tensor.matmul(ps[:], ica_sb[:, j * 64:(j + 1) * 64],
                                             ya[:, c0:c0 + 512],
                                             start=(idx == 0), stop=False)
                            idx += 1
                            nc.tensor.matmul(ps[:], icb_sb[:, j * 64:(j + 1) * 64],
                                             yb[:, c0:c0 + 512],
                                             start=False, stop=(idx == 7))
                            idx += 1
                        base = o * 1024 + pt * 512
                        if pt == 0:
                            bulk = (yt_sb[:, base + 1:base + 512], ps[:, 1:512])
                            edge = (yt_sb[:, base:base + 1], ps[:, 0:1],
                                    inv3_sb[:, 1:2])
                        else:
                            bulk = (yt_sb[:, base:base + 511], ps[:, 0:511])
                            edge = (yt_sb[:, base + 511:base + 512],
                                    ps[:, 511:512], inv3_sb[:, 2:3])
                        if (o + pt) % 2 == 0:
                            nc.scalar.activation(bulk[0], bulk[1], AF.Identity,
                                                 bias=bias_sb[:, o:o + 1],
                                                 scale=inv3_sb[:, 0:1])
                        else:
                            nc.vector.tensor_scalar(bulk[0], bulk[1],
                                                    inv3_sb[:, 0:1],
                                                    bias_sb[:, o:o + 1],
                                                    ALU.mult, ALU.add)
                        nc.vector.tensor_scalar(edge[0], edge[1], edge[2],
                                                bias_sb[:, o:o + 1],
                                                ALU.mult, ALU.add)

            nc.sync.dma_start(yt_out[:], yt_sb[:])

    nc.compile()
    return nc


def _prep_inputs(x, conditioning, w1, b1, w2, b2, bias):
    c = _consts()
    x = np.asarray(x, dtype=np.float32)
    conditioning = np.asarray(conditioning, dtype=np.float32)
    w1 = np.asarray(w1, dtype=np.float32)
    b1 = np.asarray(b1, dtype=np.float32)
    w2 = np.asarray(w2, dtype=np.float32)
    b2 = np.asarray(b2, dtype=np.float32)
    bias = np.asarray(bias, dtype=np.float32)

    w1t = w1.reshape(32, 32, 4, 64).transpose(1, 3, 2, 0).reshape(2048, 4, 32)
    w1n = np.ascontiguousarray(
        w1t.reshape(16, 128, 128).transpose(1, 0, 2).reshape(128, 2048)
    ).astype(np.float16)
    w2t = np.ascontiguousarray(w2[:, :, 0].T).astype(np.float16)      # [32, 64]
    bias64 = np.tile(bias.reshape(1, 4), (64, 1)).astype(np.float32)

    shared = {
        "eye_in": np.eye(128, dtype=np.float16),
        "w1n_in": w1n, "w2t_in": w2t, "ws_in": c["WS"],
        "wx_in": c["wx_h"], "ica_in": c["ica"], "icb_in": c["icb"],
        "b1_in": b1.reshape(32, 1).copy(),
        "b2a_in": b2[:32].reshape(32, 1).copy(),
        "b2w_in": b2[32:].reshape(32, 1).copy(),
        "inv3_in": c["inv3"], "bias_in": bias64,
    }
    in_maps = []
    for b in range(B):
        condpad = np.zeros((CI, T + K), dtype=np.float32)
        condpad[:, 128:128 + T] = conditioning[b]
        d = condpad.reshape(CI, H, 64).transpose(0, 2, 1).reshape(2048, H)
        d = np.ascontiguousarray(d.reshape(16, 128, H)).astype(np.float16)
        xp = np.pad(x[b], ((0, 0), (128, 128)), mode="reflect")
        xd = np.ascontiguousarray(
            xp.reshape(2, H, 64).transpose(0, 2, 1).reshape(2, 64, H)
            .transpose(1, 0, 2).reshape(64, 2 * H)).astype(np.float16)
        m = dict(shared)
        m["d_in"] = d
        m["xd_in"] = xd
        in_maps.append(m)
    return in_maps


def _assemble(results):
    y = np.empty((B, O, T), dtype=np.float32)
    for b in range(B):
        yt = results[b]["yt_out"]                        # [64, 4096]
        y[b] = yt.reshape(64, 4, 1024).transpose(1, 2, 0).reshape(4, T)
    return y


def kernel(x, conditioning, w1, b1, w2, b2, bias):
    from concourse.bass_utils import run_bass_kernel_spmd
    if "nc" not in _prog_cache:
        _prog_cache["nc"] = _build_program()
    nc = _prog_cache["nc"]
    in_maps = _prep_inputs(x, conditioning, w1, b1, w2, b2, bias)
    res = run_bass_kernel_spmd(nc, in_maps, core_ids=list(range(B)))
    return _assemble(res.results)


# revision 22
# speedup vs baseline: 7.2919x; 1.3043x over previous
"""DynamicSincConv1d Trainium2 kernel (v2).

Data-parallel over batch: 8 batch elements -> 8 NeuronCores. All heavy
math runs on-device; the host reshapes inputs into DMA-friendly layouts
and reassembles the output.

Key algorithmic moves vs a direct translation:
 - The windowed-sinc bank is symmetric about k=128, so every filter
   spectrum is real: the complex multiply becomes a real scaling of X
   (fbpack layout: pack a = Re fb 0..127, pack b = [fb128, Im fb 1..127]).
 - sinc(w*t) is approximated by a degree-3 polynomial in (w*t)^2
   (max err 2e-6 over the reachable |w|<=1, |t|<=0.5 range).  The
   filter synthesis + rFFT then factorizes through 4 moments
   P_j = amp*w^(2j): R = WS @ Pm with host-baked WS.  This removes the
   per-(o,i,s) sinc/sin/reciprocal elementwise work and two matmul
   layers entirely.
 - Stage-1 conv uses m=128 output tiles (4 kernel-shifts x 32 channels)
   and a cheap cross-partition j-sum, 4x fewer matmul rows.
 - iSTFT overlap-add folds into PSUM accumulation with shifted rhs
   slices; the window-square normalization is periodic except at the
   two boundary hops, so it reduces to a per-partition tensor_scalar.
 - Matmuls run in fp16/fp32r (1 cycle/row vs 4 for fp32); elementwise
   stages run in fp16 (2x DVE throughput).
"""

import math
import numpy as np

B, CI, I, O, S = 8, 32, 2, 4, 4
K, HOP, T = 256, 64, 65536
F = T // HOP + 1            # 1025 frames
H = (T + K) // HOP          # 1028
PI = math.pi
FT_F = [(0, 512), (512, 512), (1024, 1)]     # tiles of F
FT_H = [(0, 512), (512, 512), (1024, 4)]     # tiles of H

_prog_cache = {}


def _consts():
    n = np.arange(K, dtype=np.float64)
    ola = 0.5 * (1.0 - np.cos(2.0 * PI * n / K))
    fir = 0.42 - 0.5 * np.cos(2.0 * PI * n / K) + 0.08 * np.cos(4.0 * PI * n / K)

    d = np.arange(128, dtype=np.float64)
    fb = np.arange(K // 2 + 1, dtype=np.float64)
    cd = np.where(d == 0, 1.0, 2.0)
    m1full = (((-1.0) ** fb)[None, :] * cd[:, None] * fir[128 + d.astype(int)][:, None]
              / (S * K) * np.cos(2.0 * PI * np.outer(d, fb) / K))     # [128, 129]

    # sinc_n(u) ~= sum_j c_j u^(2j) on u in [0, 0.503]
    us = np.linspace(0, 0.503, 6001)
    V = np.vander(us * us, 4, increasing=True)
    c, *_ = np.linalg.lstsq(V, np.sinc(us), rcond=None)

    pw = (d[:, None] / K) ** (2 * np.arange(4)[None, :])              # [128, 4]
    M2full = np.einsum("df,dj->fj", m1full, pw) * c[None, :]          # [129, 4]
    M2a = M2full[0:128]
    M2b = np.concatenate([M2full[128:129], M2full[1:128]], axis=0)    # [128, 4]

    WS = np.zeros((128, 16, 128), dtype=np.float64)
    for oi in range(8):
        for j in range(4):
            for s in range(4):
                WS[32 * j + 4 * oi + s, oi * 2 + 0, :] = M2a[:, j]
                WS[32 * j + 4 * oi + s, oi * 2 + 1, :] = M2b[:, j]
    WS = np.ascontiguousarray(WS.reshape(128, 2048)).astype(np.float16)

    kk = np.arange(K, dtype=np.float64)
    ang = 2.0 * PI * np.outer(kk, fb) / K
    wre = ola[:, None] * np.cos(ang)
    wim = -ola[:, None] * np.sin(ang)
    colsA = wre[:, 0:128]
    colsB = np.concatenate([wre[:, 128:129], wim[:, 1:128]], axis=1)
    wx_full = np.concatenate([colsA, colsB], axis=1).reshape(4, 64, 256)
    wx_h = np.ascontiguousarray(
        wx_full.transpose(1, 0, 2).reshape(64, 1024)).astype(np.float16)

    cp = np.where(fb == 0, 1.0, 2.0)
    icre = (cp[:, None] / K) * np.cos(2.0 * PI * np.outer(fb, n) / K) * ola[None, :]
    icim = (-2.0 / K) * np.sin(2.0 * PI * np.outer(fb, n) / K) * ola[None, :]
    ica = np.ascontiguousarray(icre[0:128]).astype(np.float16)        # [128, 256]
    icb = np.concatenate([icre[128:129], icim[1:128]], axis=0).astype(np.float16)

    ola2 = ola * ola
    r = np.arange(64)
    env_int = sum(ola2[64 * j + r] for j in range(4))
    env_p0 = sum(ola2[64 * j + r] for j in (0, 1, 2))
    env_p1023 = sum(ola2[64 * j + r] for j in (1, 2, 3))
    inv3 = np.stack([1.0 / env_int, 1.0 / env_p0, 1.0 / env_p1023],
                    axis=1).astype(np.float32)                        # [64, 3]

    return dict(WS=WS, wx_h=wx_h, ica=ica, icb=icb, inv3=inv3)


def _build_program():
    import concourse.bacc as bacc
    import concourse.mybir as mybir
    import concourse.tile as tile

    f32 = mybir.dt.float32
    f16 = mybir.dt.float16
    f32r = mybir.dt.float32r
    AF = mybir.ActivationFunctionType
    ALU = mybir.AluOpType

    nc = bacc.Bacc("TRN2", target_bir_lowering=False, debug=False, num_devices=8)

    eye_in = nc.dram_tensor("eye_in", [128, 128], f16, kind="ExternalInput")
    d_in = nc.dram_tensor("d_in", [16, 128, H], f16, kind="ExternalInput")
    xd_in = nc.dram_tensor("xd_in", [64, 2 * H], f16, kind="ExternalInput")
    w1n_in = nc.dram_tensor("w1n_in", [128, 2048], f16, kind="ExternalInput")
    w2t_in = nc.dram_tensor("w2t_in", [32, 64], f16, kind="ExternalInput")
    ws_in = nc.dram_tensor("ws_in", [128, 2048], f16, kind="ExternalInput")
    wx_in = nc.dram_tensor("wx_in", [64, 1024], f16, kind="ExternalInput")
    ica_in = nc.dram_tensor("ica_in", [128, 256], f16, kind="ExternalInput")
    icb_in = nc.dram_tensor("icb_in", [128, 256], f16, kind="ExternalInput")
    b1_in = nc.dram_tensor("b1_in", [32, 1], f32, kind="ExternalInput")
    b1s_in = nc.dram_tensor("b1s_in", [32, 1], f32, kind="ExternalInput")
    b2a_in = nc.dram_tensor("b2a_in", [32, 1], f32, kind="ExternalInput")
    b2w_in = nc.dram_tensor("b2w_in", [32, 1], f32, kind="ExternalInput")
    inv3_in = nc.dram_tensor("inv3_in", [64, 3], f32, kind="ExternalInput")
    bias_in = nc.dram_tensor("bias_in", [64, 4], f32, kind="ExternalInput")
    yt_out = nc.dram_tensor("yt_out", [64, 4096], f32, kind="ExternalOutput")

    with tile.TileContext(nc) as tc:
        with tc.tile_pool(name="cpool", bufs=1) as cpool:
            w1n_sb = cpool.tile([128, 2048], f16, tag="w1n")
            w2t_sb = cpool.tile([32, 64], f16, tag="w2t")
            ws_sb = cpool.tile([128, 2048], f16, tag="ws")
            wx_sb = cpool.tile([64, 1024], f16, tag="wx")
            ica_sb = cpool.tile([128, 256], f16, tag="ica")
            icb_sb = cpool.tile([128, 256], f16, tag="icb")
            b1_sb = cpool.tile([32, 1], f32, tag="b1")
            b1s_sb = cpool.tile([32, 1], f32, tag="b1s")
            b2a_sb = cpool.tile([32, 1], f32, tag="b2a")
            b2w_sb = cpool.tile([32, 1], f32, tag="b2w")
            inv3_sb = cpool.tile([64, 3], f32, tag="inv3")
            bias_sb = cpool.tile([64, 4], f32, tag="bias")
            xd_sb = cpool.tile([64, 2 * H], f16, tag="xd")
            xa_sb = cpool.tile([128, 2 * F], f16, tag="xa")
            xb_sb = cpool.tile([128, 2 * F], f16, tag="xb")
            eye_sb = cpool.tile([128, 128], f16, tag="eye")
            wid_sb = cpool.tile([32, F], f16, tag="wid")
            wsq_sb = cpool.tile([32, F], f16, tag="wsq")
            w4_sb = cpool.tile([32, F], f16, tag="w4")
            pm_sb = cpool.tile([128, F], f16, tag="pm")
            p1_sb = cpool.tile([32, F], f16, tag="p1")
            p2_sb = cpool.tile([32, F], f16, tag="p2")
            p3_sb = cpool.tile([32, F], f16, tag="p3")
            outs_sb = cpool.tile([128, H], f16, tag="outs")
            hb_sb = cpool.tile([32, F], f32, tag="hb")
            lt_sb = cpool.tile([32, F], f32, tag="lt")
            h_sb = cpool.tile([32, F], f16, tag="h")
            fpa_sb = cpool.tile([128, 8 * F], f16, tag="fpa")
            fpb_sb = cpool.tile([128, 8 * F], f16, tag="fpb")
            yt_sb = cpool.tile([64, 4096], f32, tag="yt")

            for t_sb, t_in in ((wx_sb, wx_in), (xd_sb, xd_in), (w1n_sb, w1n_in),
                               (w2t_sb, w2t_in), (ws_sb, ws_in), (eye_sb, eye_in),
                               (ica_sb, ica_in), (icb_sb, icb_in),
                               (b1_sb, b1_in), (b1s_sb, b1s_in),
                               (b2a_sb, b2a_in), (b2w_sb, b2w_in),
                               (inv3_sb, inv3_in), (bias_sb, bias_in)):
                nc.sync.dma_start(t_sb[:], t_in[:])

            with tc.tile_pool(name="dpool", bufs=1) as dpool:
                dts = []
                for c in range(16):
                    dtile = dpool.tile([128, H], f16, tag=f"d{c}")
                    nc.sync.dma_start(dtile[:], d_in[c])
                    dts.append(dtile)

                # STFT quarters double as PE filler between dependent stages
                with tc.tile_pool(name="ps4", bufs=1, space="PSUM") as ps4:
                    def stft_quarter(i, mt, xdst):
                        ps = ps4.tile([128, F], f32, tag="ps4", name=f"ps4_{i}{mt}")
                        for (f0, nf) in FT_F:
                            for j in range(4):
                                nc.tensor.matmul(
                                    ps[:, f0:f0 + nf],
                                    wx_sb[:, j * 256 + mt * 128:
                                          j * 256 + mt * 128 + 128],
                                    xd_sb[:, i * H + f0 + j:i * H + f0 + j + nf],
                                    start=(j == 0), stop=(j == 3))
                        nc.scalar.activation(xdst[:, i * F:(i + 1) * F],
                                             ps[:], AF.Copy)

                    stft_quarter(0, 0, xa_sb)

                    # ---- stage 1: conditioning conv, m=128 (4j x 32ch) ----
                    with tc.tile_pool(name="ps1", bufs=1, space="PSUM") as ps1:
                        out_ps = ps1.tile([128, H], f32, tag="out1")
                        for c in range(16):
                            for (f0, nf) in FT_H:
                                nc.tensor.matmul(
                                    out_ps[:, f0:f0 + nf],
                                    w1n_sb[:, c * 128:(c + 1) * 128],
                                    dts[c][:, f0:f0 + nf],
                                    start=(c == 0), stop=(c == 15))
                        nc.scalar.activation(outs_sb[:, 0:516],
                                             out_ps[:, 0:516], AF.Copy)
                        nc.scalar.activation(outs_sb[:, 516:H],
                                             out_ps[:, 516:H], AF.Copy)

                    stft_quarter(0, 1, xb_sb)

                    # j-sum via 4 shifted accumulating matmuls (identity
                    # lhsT), then bias + leaky_relu
                    with tc.tile_pool(name="psh", bufs=1, space="PSUM") as psh:
                        h_ps = psh.tile([32, F], f32, tag="hps")
                        for (f0, nf) in FT_F:
                            for j in range(4):
                                nc.tensor.matmul(
                                    h_ps[:, f0:f0 + nf],
                                    eye_sb[:, j * 32:(j + 1) * 32],
                                    outs_sb[:, f0 + j:f0 + j + nf],
                                    start=(j == 0), stop=(j == 3))
                        nc.scalar.activation(hb_sb[:], h_ps[:], AF.Identity,
                                             bias=b1_sb[:, 0:1])
                        nc.scalar.activation(lt_sb[:], h_ps[:], AF.Identity,
                                             bias=b1s_sb[:, 0:1], scale=0.01)
                        nc.vector.tensor_max(h_sb[:], hb_sb[:], lt_sb[:])

                    stft_quarter(1, 0, xa_sb)

                    # ---- stage 2: 1x1 conv + tanh -> amp / wid ----
                    with tc.tile_pool(name="ps2", bufs=2, space="PSUM") as ps2:
                        for (f0, nf) in FT_F:
                            pa = ps2.tile([32, nf], f32, tag="ps2a")
                            nc.tensor.matmul(pa[:], w2t_sb[:, 0:32],
                                             h_sb[:, f0:f0 + nf],
                                             start=True, stop=True)
                            nc.scalar.activation(pm_sb[0:32, f0:f0 + nf], pa[:],
                                                 AF.Tanh, bias=b2a_sb[:, 0:1])
                            pw = ps2.tile([32, nf], f32, tag="ps2w")
                            nc.tensor.matmul(pw[:], w2t_sb[:, 32:64],
                                             h_sb[:, f0:f0 + nf],
                                             start=True, stop=True)
                            nc.scalar.activation(wid_sb[:, f0:f0 + nf], pw[:],
                                                 AF.Tanh, bias=b2w_sb[:, 0:1])

                    # ---- moments: pm rows = [amp, amp*w^2, amp*w^4, amp*w^6]
                    # computed on partitions 0:32, placed via SBUF DMAs
                    nc.vector.tensor_mul(wsq_sb[:], wid_sb[:], wid_sb[:])
                    nc.vector.tensor_mul(p1_sb[:], pm_sb[0:32, :], wsq_sb[:])
                    nc.vector.tensor_mul(w4_sb[:], wsq_sb[:], wsq_sb[:])
                    nc.vector.tensor_mul(p2_sb[:], pm_sb[0:32, :], w4_sb[:])
                    nc.vector.tensor_mul(p3_sb[:], p1_sb[:], w4_sb[:])
                    nc.sync.dma_start(pm_sb[32:64, :], p1_sb[:])
                    nc.sync.dma_start(pm_sb[64:96, :], p2_sb[:])
                    nc.sync.dma_start(pm_sb[96:128, :], p3_sb[:])

                    stft_quarter(1, 1, xb_sb)

            # ---- stage 3: R = WS @ Pm per (oi, pack) -> fpa/fpb f16 ----
            with tc.tile_pool(name="ps3", bufs=2, space="PSUM") as ps3:
                for oi in range(8):
                    for (p, dest) in ((0, fpa_sb), (1, fpb_sb)):
                        ps = ps3.tile([128, F], f32, tag="ps3")
                        for (f0, nf) in FT_F:
                            nc.tensor.matmul(
                                ps[:, f0:f0 + nf],
                                ws_sb[:, (oi * 2 + p) * 128:(oi * 2 + p + 1) * 128],
                                pm_sb[:, f0:f0 + nf],
                                start=True, stop=True)
                        dst = dest[:, oi * F:(oi + 1) * F]
                        if p == 0 or oi < 5:
                            nc.scalar.activation(dst, ps[:], AF.Copy)
                        else:
                            nc.vector.tensor_scalar(dst, ps[:], 1.0, None, ALU.mult)

            # ---- stage 5+6 per o: cmul, iSTFT with OLA in PSUM ----
            with tc.tile_pool(name="yp", bufs=2) as ypool, \
                 tc.tile_pool(name="ctp", bufs=2) as ctpool, \
                 tc.tile_pool(name="ps6", bufs=4, space="PSUM") as ps6:
                for o in range(4):
                    ya = ypool.tile([128, H], f16, tag="ya")
                    yb = ypool.tile([128, H], f16, tag="yb")
                    ta_t = ctpool.tile([128, 2 * F], f16, tag="cta")
                    tb_t = ctpool.tile([128, 2 * F], f16, tag="ctb")
                    o2 = 2 * o
                    nc.gpsimd.memset(ya[:, 0:1], 0.0)
                    nc.gpsimd.memset(ya[:, 1026:1028], 0.0)
                    nc.gpsimd.memset(yb[:, 0:1], 0.0)
                    nc.gpsimd.memset(yb[:, 1026:1028], 0.0)
                    nc.vector.tensor_mul(ta_t[:], xa_sb[:],
                                         fpa_sb[:, o2 * F:(o2 + 2) * F])
                    nc.gpsimd.tensor_add(ya[:, 1:1 + F], ta_t[:, 0:F],
                                         ta_t[:, F:2 * F])
                    nc.vector.tensor_mul(tb_t[:], xb_sb[:],
                                         fpb_sb[:, o2 * F:(o2 + 2) * F])
                    nc.gpsimd.tensor_add(yb[:, 1:1 + F], tb_t[:, 0:F],
                                         tb_t[:, F:2 * F])

                    for pt in range(2):
                        ps = ps6.tile([64, 512], f32, tag="ps6")
                        idx = 0
                        for j in range(4):
                            c0 = pt * 512 + 3 - j
                            nc.tensor.matmul(ps[:], ica_sb[:, j * 64:(j + 1) * 64],
                                             ya[:, c0:c0 + 512],
                                             start=(idx == 0), stop=False)
                            idx += 1
                            nc.tensor.matmul(ps[:], icb_sb[:, j * 64:(j + 1) * 64],
                                             yb[:, c0:c0 + 512],
                                             start=False, stop=(idx == 7))
                            idx += 1
                        base = o * 1024 + pt * 512
                        if pt == 0:
                            bulk = (yt_sb[:, base + 1:base + 512], ps[:, 1:512])
                            edge = (yt_sb[:, base:base + 1], ps[:, 0:1],
                                    inv3_sb[:, 1:2])
                        else:
                            bulk = (yt_sb[:, base:base + 511], ps[:, 0:511])
                            edge = (yt_sb[:, base + 511:base + 512],
                                    ps[:, 511:512], inv3_sb[:, 2:3])
                        nc.scalar.activation(bulk[0], bulk[1], AF.Identity,
                                             bias=bias_sb[:, o:o + 1],
                                             scale=inv3_sb[:, 0:1])
                        nc.vector.tensor_scalar(edge[0], edge[1], edge[2],
                                                bias_sb[:, o:o + 1],
                                                ALU.mult, ALU.add)

            nc.sync.dma_start(yt_out[:], yt_sb[:])

    nc.compile()
    return nc


def _prep_inputs(x, conditioning, w1, b1, w2, b2, bias):
    c = _consts()
    x = np.asarray(x, dtype=np.float32)
    conditioning = np.asarray(conditioning, dtype=np.float32)
    w1 = np.asarray(w1, dtype=np.float32)
    b1 = np.asarray(b1, dtype=np.float32)
    w2 = np.asarray(w2, dtype=np.float32)
    b2 = np.asarray(b2, dtype=np.float32)
    bias = np.asarray(bias, dtype=np.float32)

    w1t = w1.reshape(32, 32, 4, 64).transpose(1, 3, 2, 0).reshape(2048, 4, 32)
    w1n = np.ascontiguousarray(
        w1t.reshape(16, 128, 128).transpose(1, 0, 2).reshape(128, 2048)
    ).astype(np.float16)
    w2t = np.ascontiguousarray(w2[:, :, 0].T).astype(np.float16)      # [32, 64]
    bias64 = np.tile(bias.reshape(1, 4), (64, 1)).astype(np.float32)

    shared = {
        "eye_in": np.eye(128, dtype=np.float16),
        "w1n_in": w1n, "w2t_in": w2t, "ws_in": c["WS"],
        "wx_in": c["wx_h"], "ica_in": c["ica"], "icb_in": c["icb"],
        "b1_in": b1.reshape(32, 1).copy(),
        "b1s_in": (0.01 * b1).reshape(32, 1).copy(),
        "b2a_in": b2[:32].reshape(32, 1).copy(),
        "b2w_in": b2[32:].reshape(32, 1).copy(),
        "inv3_in": c["inv3"], "bias_in": bias64,
    }
    in_maps = []
    for b in range(B):
        condpad = np.zeros((CI, T + K), dtype=np.float32)
        condpad[:, 128:128 + T] = conditioning[b]
        d = condpad.reshape(CI, H, 64).transpose(0, 2, 1).reshape(2048, H)
        d = np.ascontiguousarray(d.reshape(16, 128, H)).astype(np.float16)
        xp = np.pad(x[b], ((0, 0), (128, 128)), mode="reflect")
        xd = np.ascontiguousarray(
            xp.reshape(2, H, 64).transpose(0, 2, 1).reshape(2, 64, H)
            .transpose(1, 0, 2).reshape(64, 2 * H)).astype(np.float16)
        m = dict(shared)
        m["d_in"] = d
        m["xd_in"] = xd
        in_maps.append(m)
    return in_maps


def _assemble(results):
    y = np.empty((B, O, T), dtype=np.float32)
    for b in range(B):
        yt = results[b]["yt_out"]                        # [64, 4096]
        y[b] = yt.reshape(64, 4, 1024).transpose(1, 2, 0).reshape(4, T)
    return y


def kernel(x, conditioning, w1, b1, w2, b2, bias):
    from concourse.bass_utils import run_bass_kernel_spmd
    if "nc" not in _prog_cache:
        _prog_cache["nc"] = _build_program()
    nc = _prog_cache["nc"]
    in_maps = _prep_inputs(x, conditioning, w1, b1, w2, b2, bias)
    res = run_bass_kernel_spmd(nc, in_maps, core_ids=list(range(B)))
    return _assemble(res.results)


# revision 28
# speedup vs baseline: 7.8711x; 1.0794x over previous
"""DynamicSincConv1d Trainium2 kernel (v2).

Data-parallel over batch: 8 batch elements -> 8 NeuronCores. All heavy
math runs on-device; the host reshapes inputs into DMA-friendly layouts
and reassembles the output.

Key algorithmic moves vs a direct translation:
 - The windowed-sinc bank is symmetric about k=128, so every filter
   spectrum is real: the complex multiply becomes a real scaling of X
   (fbpack layout: pack a = Re fb 0..127, pack b = [fb128, Im fb 1..127]).
 - sinc(w*t) is approximated by a degree-3 polynomial in (w*t)^2
   (max err 2e-6 over the reachable |w|<=1, |t|<=0.5 range).  The
   filter synthesis + rFFT then factorizes through 4 moments
   P_j = amp*w^(2j): R = WS @ Pm with host-baked WS.  This removes the
   per-(o,i,s) sinc/sin/reciprocal elementwise work and two matmul
   layers entirely.
 - Stage-1 conv uses m=128 output tiles (4 kernel-shifts x 32 channels)
   and a cheap cross-partition j-sum, 4x fewer matmul rows.
 - iSTFT overlap-add folds into PSUM accumulation with shifted rhs
   slices; the window-square normalization is periodic except at the
   two boundary hops, so it reduces to a per-partition tensor_scalar.
 - Matmuls run in fp16/fp32r (1 cycle/row vs 4 for fp32); elementwise
   stages run in fp16 (2x DVE throughput).
"""

import math
import numpy as np

B, CI, I, O, S = 8, 32, 2, 4, 4
K, HOP, T = 256, 64, 65536
F = T // HOP + 1            # 1025 frames
H = (T + K) // HOP          # 1028
PI = math.pi
FT_F = [(0, 512), (512, 512), (1024, 1)]     # tiles of F
FT_H = [(0, 512), (512, 512), (1024, 4)]     # tiles of H

_prog_cache = {}


def _consts():
    n = np.arange(K, dtype=np.float64)
    ola = 0.5 * (1.0 - np.cos(2.0 * PI * n / K))
    fir = 0.42 - 0.5 * np.cos(2.0 * PI * n / K) + 0.08 * np.cos(4.0 * PI * n / K)

    d = np.arange(128, dtype=np.float64)
    fb = np.arange(K // 2 + 1, dtype=np.float64)
    cd = np.where(d == 0, 1.0, 2.0)
    m1full = (((-1.0) ** fb)[None, :] * cd[:, None] * fir[128 + d.astype(int)][:, None]
              / (S * K) * np.cos(2.0 * PI * np.outer(d, fb) / K))     # [128, 129]

    # sinc_n(u) ~= sum_j c_j u^(2j) on u in [0, 0.503]
    us = np.linspace(0, 0.503, 6001)
    V = np.vander(us * us, 4, increasing=True)
    c, *_ = np.linalg.lstsq(V, np.sinc(us), rcond=None)

    pw = (d[:, None] / K) ** (2 * np.arange(4)[None, :])              # [128, 4]
    M2full = np.einsum("df,dj->fj", m1full, pw) * c[None, :]          # [129, 4]
    M2a = M2full[0:128]
    M2b = np.concatenate([M2full[128:129], M2full[1:128]], axis=0)    # [128, 4]

    WS = np.zeros((128, 16, 128), dtype=np.float64)
    for oi in range(8):
        for j in range(4):
            for s in range(4):
                WS[32 * j + 4 * oi + s, oi * 2 + 0, :] = M2a[:, j]
                WS[32 * j + 4 * oi + s, oi * 2 + 1, :] = M2b[:, j]
    WS = np.ascontiguousarray(WS.reshape(128, 2048)).astype(np.float16)

    kk = np.arange(K, dtype=np.float64)
    ang = 2.0 * PI * np.outer(kk, fb) / K
    wre = ola[:, None] * np.cos(ang)
    wim = -ola[:, None] * np.sin(ang)
    colsA = wre[:, 0:128]
    colsB = np.concatenate([wre[:, 128:129], wim[:, 1:128]], axis=1)
    wx_full = np.concatenate([colsA, colsB], axis=1).reshape(4, 64, 256)
    wx_h = np.ascontiguousarray(
        wx_full.transpose(1, 0, 2).reshape(64, 1024)).astype(np.float16)

    cp = np.where(fb == 0, 1.0, 2.0)
    icre = (cp[:, None] / K) * np.cos(2.0 * PI * np.outer(fb, n) / K) * ola[None, :]
    icim = (-2.0 / K) * np.sin(2.0 * PI * np.outer(fb, n) / K) * ola[None, :]
    ica = np.ascontiguousarray(icre[0:128]).astype(np.float16)        # [128, 256]
    icb = np.concatenate([icre[128:129], icim[1:128]], axis=0).astype(np.float16)

    ola2 = ola * ola
    r = np.arange(64)
    env_int = sum(ola2[64 * j + r] for j in range(4))
    env_p0 = sum(ola2[64 * j + r] for j in (0, 1, 2))
    env_p1023 = sum(ola2[64 * j + r] for j in (1, 2, 3))
    inv3 = np.stack([1.0 / env_int, 1.0 / env_p0, 1.0 / env_p1023],
                    axis=1).astype(np.float32)                        # [64, 3]

    return dict(WS=WS, wx_h=wx_h, ica=ica, icb=icb, inv3=inv3)


def _build_program():
    import concourse.bacc as bacc
    import concourse.mybir as mybir
    import concourse.tile as tile

    f32 = mybir.dt.float32
    f16 = mybir.dt.float16
    f32r = mybir.dt.float32r
    AF = mybir.ActivationFunctionType
    ALU = mybir.AluOpType

    nc = bacc.Bacc("TRN2", target_bir_lowering=False, debug=False, num_devices=8)

    eye_in = nc.dram_tensor("eye_in", [128, 128], f16, kind="ExternalInput")
    d_in = nc.dram_tensor("d_in", [16, 128, H], f16, kind="ExternalInput")
    xd_in = nc.dram_tensor("xd_in", [64, 2 * H], f16, kind="ExternalInput")
    w1n_in = nc.dram_tensor("w1n_in", [128, 2048], f16, kind="ExternalInput")
    w2r_in = nc.dram_tensor("w2r_in", [32, 256], f16, kind="ExternalInput")
    ws_in = nc.dram_tensor("ws_in", [128, 2048], f16, kind="ExternalInput")
    wx_in = nc.dram_tensor("wx_in", [64, 1024], f16, kind="ExternalInput")
    ica_in = nc.dram_tensor("ica_in", [128, 256], f16, kind="ExternalInput")
    icb_in = nc.dram_tensor("icb_in", [128, 256], f16, kind="ExternalInput")
    b1_in = nc.dram_tensor("b1_in", [32, 1], f32, kind="ExternalInput")
    b1s_in = nc.dram_tensor("b1s_in", [32, 1], f32, kind="ExternalInput")
    b2a_in = nc.dram_tensor("b2a_in", [128, 1], f32, kind="ExternalInput")
    b2w_in = nc.dram_tensor("b2w_in", [128, 1], f32, kind="ExternalInput")
    mk_in = nc.dram_tensor("mk_in", [128, 4], f32, kind="ExternalInput")
    inv3_in = nc.dram_tensor("inv3_in", [64, 3], f32, kind="ExternalInput")
    bias_in = nc.dram_tensor("bias_in", [64, 4], f32, kind="ExternalInput")
    yt_out = nc.dram_tensor("yt_out", [64, 4096], f32, kind="ExternalOutput")

    with tile.TileContext(nc) as tc:
        with tc.tile_pool(name="cpool", bufs=1) as cpool:
            w1n_sb = cpool.tile([128, 2048], f16, tag="w1n")
            w2r_sb = cpool.tile([32, 256], f16, tag="w2r")
            ws_sb = cpool.tile([128, 2048], f16, tag="ws")
            wx_sb = cpool.tile([64, 1024], f16, tag="wx")
            ica_sb = cpool.tile([128, 256], f16, tag="ica")
            icb_sb = cpool.tile([128, 256], f16, tag="icb")
            b1_sb = cpool.tile([32, 1], f32, tag="b1")
            b1s_sb = cpool.tile([32, 1], f32, tag="b1s")
            b2a_sb = cpool.tile([128, 1], f32, tag="b2a")
            b2w_sb = cpool.tile([128, 1], f32, tag="b2w")
            mk_sb = cpool.tile([128, 4], f32, tag="mk")
            inv3_sb = cpool.tile([64, 3], f32, tag="inv3")
            bias_sb = cpool.tile([64, 4], f32, tag="bias")
            xd_sb = cpool.tile([64, 2 * H], f16, tag="xd")
            xa_sb = cpool.tile([128, 2 * F], f16, tag="xa")
            xb_sb = cpool.tile([128, 2 * F], f16, tag="xb")
            eye_sb = cpool.tile([128, 128], f16, tag="eye")
            wid4_sb = cpool.tile([128, F], f16, tag="wid4")
            wsq4_sb = cpool.tile([128, F], f16, tag="wsq4")
            mb1_sb = cpool.tile([128, F], f16, tag="mb1")
            mb2_sb = cpool.tile([128, F], f16, tag="mb2")
            b2sq_sb = cpool.tile([128, F], f16, tag="b2sq")
            tm_sb = cpool.tile([128, F], f16, tag="tm")
            pm_sb = cpool.tile([128, F], f16, tag="pm")
            outs_sb = cpool.tile([128, H], f16, tag="outs")
            hb_sb = cpool.tile([32, F], f32, tag="hb")
            lt_sb = cpool.tile([32, F], f32, tag="lt")
            h_sb = cpool.tile([32, F], f16, tag="h")
            fpa_sb = cpool.tile([128, 8 * F], f16, tag="fpa")
            fpb_sb = cpool.tile([128, 8 * F], f16, tag="fpb")
            yt_sb = cpool.tile([64, 4096], f32, tag="yt")

            # DMA spread: sync + scalar are the HW DGE queues, gpsimd is
            # the software queue.  STFT inputs first on sync.
            nc.sync.dma_start(wx_sb[:], wx_in[:])
            nc.sync.dma_start(xd_sb[:], xd_in[:])
            nc.scalar.dma_start(w1n_sb[:], w1n_in[:])
            nc.scalar.dma_start(ws_sb[:], ws_in[:])
            for t_sb, t_in in ((w2r_sb, w2r_in), (eye_sb, eye_in),
                               (ica_sb, ica_in), (icb_sb, icb_in),
                               (b1_sb, b1_in), (b1s_sb, b1s_in),
                               (b2a_sb, b2a_in), (b2w_sb, b2w_in),
                               (mk_sb, mk_in),
                               (inv3_sb, inv3_in), (bias_sb, bias_in)):
                nc.scalar.dma_start(t_sb[:], t_in[:])

            with tc.tile_pool(name="dpool", bufs=1) as dpool:
                dts = []
                for c in range(16):
                    dtile = dpool.tile([128, H], f16, tag=f"d{c}")
                    eng = nc.sync if c % 2 == 0 else nc.scalar
                    eng.dma_start(dtile[:], d_in[c])
                    dts.append(dtile)

                # STFT quarters double as PE filler between dependent stages
                with tc.tile_pool(name="ps4", bufs=1, space="PSUM") as ps4:
                    def stft_quarter(i, mt, xdst):
                        ps = ps4.tile([128, F], f32, tag="ps4", name=f"ps4_{i}{mt}")
                        for (f0, nf) in FT_F:
                            for j in range(4):
                                nc.tensor.matmul(
                                    ps[:, f0:f0 + nf],
                                    wx_sb[:, j * 256 + mt * 128:
                                          j * 256 + mt * 128 + 128],
                                    xd_sb[:, i * H + f0 + j:i * H + f0 + j + nf],
                                    start=(j == 0), stop=(j == 3))
                        nc.scalar.activation(xdst[:, i * F:(i + 1) * F],
                                             ps[:], AF.Copy)

                    stft_quarter(0, 0, xa_sb)

                    # ---- stage 1: conditioning conv, m=128 (4j x 32ch) ----
                    with tc.tile_pool(name="ps1", bufs=1, space="PSUM") as ps1:
                        out_ps = ps1.tile([128, H], f32, tag="out1")
                        for c in range(16):
                            for (f0, nf) in FT_H:
                                nc.tensor.matmul(
                                    out_ps[:, f0:f0 + nf],
                                    w1n_sb[:, c * 128:(c + 1) * 128],
                                    dts[c][:, f0:f0 + nf],
                                    start=(c == 0), stop=(c == 15))
                        nc.scalar.activation(outs_sb[:, 0:516],
                                             out_ps[:, 0:516], AF.Copy)
                        nc.scalar.activation(outs_sb[:, 516:H],
                                             out_ps[:, 516:H], AF.Copy)

                    stft_quarter(0, 1, xb_sb)

                    # j-sum via 4 shifted accumulating matmuls (identity
                    # lhsT), then bias + leaky_relu
                    with tc.tile_pool(name="psh", bufs=1, space="PSUM") as psh:
                        h_ps = psh.tile([32, F], f32, tag="hps")
                        for (f0, nf) in FT_F:
                            for j in range(4):
                                nc.tensor.matmul(
                                    h_ps[:, f0:f0 + nf],
                                    eye_sb[:, j * 32:(j + 1) * 32],
                                    outs_sb[:, f0 + j:f0 + j + nf],
                                    start=(j == 0), stop=(j == 3))
                        nc.scalar.activation(hb_sb[:], h_ps[:], AF.Identity,
                                             bias=b1_sb[:, 0:1])
                        nc.scalar.activation(lt_sb[:], h_ps[:], AF.Identity,
                                             bias=b1s_sb[:, 0:1], scale=0.01)
                        nc.vector.tensor_max(h_sb[:], hb_sb[:], lt_sb[:])

                    stft_quarter(1, 0, xa_sb)

                    # ---- stage 2: 1x1 conv (outputs replicated 4x over
                    # partition blocks) + tanh -> amp4 (in pm) / wid4 ----
                    with tc.tile_pool(name="ps2", bufs=2, space="PSUM") as ps2:
                        for (f0, nf) in FT_F:
                            pa = ps2.tile([128, nf], f32, tag="ps2a")
                            nc.tensor.matmul(pa[:], w2r_sb[:, 0:128],
                                             h_sb[:, f0:f0 + nf],
                                             start=True, stop=True)
                            nc.scalar.activation(pm_sb[:, f0:f0 + nf], pa[:],
                                                 AF.Tanh, bias=b2a_sb[:, 0:1])
                            pw = ps2.tile([128, nf], f32, tag="ps2w")
                            nc.tensor.matmul(pw[:], w2r_sb[:, 128:256],
                                             h_sb[:, f0:f0 + nf],
                                             start=True, stop=True)
                            nc.scalar.activation(wid4_sb[:, f0:f0 + nf], pw[:],
                                                 AF.Tanh, bias=b2w_sb[:, 0:1])

                    # ---- moments in place: pm = amp * [1,w2,w4,w6] by
                    # partition block via per-partition mask scalars:
                    # mb1 = [1,w2,1,w2], mb2 = [1,1,w2,w2], pm = amp*mb1*mb2^2
                    nc.vector.tensor_mul(wsq4_sb[:], wid4_sb[:], wid4_sb[:])
                    nc.vector.tensor_scalar(mb1_sb[:], wsq4_sb[:],
                                            mk_sb[:, 0:1], mk_sb[:, 1:2],
                                            ALU.mult, ALU.add)
                    nc.vector.tensor_scalar(mb2_sb[:], wsq4_sb[:],
                                            mk_sb[:, 2:3], mk_sb[:, 3:4],
                                            ALU.mult, ALU.add)
                    nc.gpsimd.tensor_mul(b2sq_sb[:], mb2_sb[:], mb2_sb[:])
                    nc.vector.tensor_mul(tm_sb[:], pm_sb[:], mb1_sb[:])
                    nc.vector.tensor_mul(pm_sb[:], tm_sb[:], b2sq_sb[:])

                    stft_quarter(1, 1, xb_sb)

            # ---- stage 3: R = WS @ Pm per (oi, pack) -> fpa/fpb f16 ----
            with tc.tile_pool(name="ps3", bufs=2, space="PSUM") as ps3:
                for oi in range(8):
                    for (p, dest) in ((0, fpa_sb), (1, fpb_sb)):
                        ps = ps3.tile([128, F], f32, tag="ps3")
                        for (f0, nf) in FT_F:
                            nc.tensor.matmul(
                                ps[:, f0:f0 + nf],
                                ws_sb[:, (oi * 2 + p) * 128:(oi * 2 + p + 1) * 128],
                                pm_sb[:, f0:f0 + nf],
                                start=True, stop=True)
                        dst = dest[:, oi * F:(oi + 1) * F]
                        if p == 0 or oi < 5:
                            nc.scalar.activation(dst, ps[:], AF.Copy)
                        else:
                            nc.vector.tensor_scalar(dst, ps[:], 1.0, None, ALU.mult)

            # ---- stage 5+6 per o: cmul, iSTFT with OLA in PSUM ----
            with tc.tile_pool(name="yp", bufs=2) as ypool, \
                 tc.tile_pool(name="ctp", bufs=2) as ctpool, \
                 tc.tile_pool(name="ps6", bufs=4, space="PSUM") as ps6:
                for o in range(4):
                    ya = ypool.tile([128, H], f16, tag="ya")
                    yb = ypool.tile([128, H], f16, tag="yb")
                    ta_t = ctpool.tile([128, 2 * F], f16, tag="cta")
                    tb_t = ctpool.tile([128, 2 * F], f16, tag="ctb")
                    o2 = 2 * o
                    nc.gpsimd.memset(ya[:, 0:1], 0.0)
                    nc.gpsimd.memset(ya[:, 1026:1028], 0.0)
                    nc.gpsimd.memset(yb[:, 0:1], 0.0)
                    nc.gpsimd.memset(yb[:, 1026:1028], 0.0)
                    nc.vector.tensor_mul(ta_t[:], xa_sb[:],
                                         fpa_sb[:, o2 * F:(o2 + 2) * F])
                    nc.gpsimd.tensor_add(ya[:, 1:1 + F], ta_t[:, 0:F],
                                         ta_t[:, F:2 * F])
                    nc.vector.tensor_mul(tb_t[:], xb_sb[:],
                                         fpb_sb[:, o2 * F:(o2 + 2) * F])
                    nc.gpsimd.tensor_add(yb[:, 1:1 + F], tb_t[:, 0:F],
                                         tb_t[:, F:2 * F])

                    for pt in range(2):
                        ps = ps6.tile([64, 512], f32, tag="ps6")
                        idx = 0
                        for j in range(4):
                            c0 = pt * 512 + 3 - j
                            nc.tensor.matmul(ps[:], ica_sb[:, j * 64:(j + 1) * 64],
                                             ya[:, c0:c0 + 512],
                                             start=(idx == 0), stop=False)
                            idx += 1
                            nc.tensor.matmul(ps[:], icb_sb[:, j * 64:(j + 1) * 64],
                                             yb[:, c0:c0 + 512],
                                             start=False, stop=(idx == 7))
                            idx += 1
                        base = o * 1024 + pt * 512
                        if pt == 0:
                            bulk = (yt_sb[:, base + 1:base + 512], ps[:, 1:512])
                            edge = (yt_sb[:, base:base + 1], ps[:, 0:1],
                                    inv3_sb[:, 1:2])
                        else:
                            bulk = (yt_sb[:, base:base + 511], ps[:, 0:511])
                            edge = (yt_sb[:, base + 511:base + 512],
                                    ps[:, 511:512], inv3_sb[:, 2:3])
                        nc.scalar.activation(bulk[0], bulk[1], AF.Identity,
                                             bias=bias_sb[:, o:o + 1],
                                             scale=inv3_sb[:, 0:1])
                        nc.vector.tensor_scalar(edge[0], edge[1], edge[2],
                                                bias_sb[:, o:o + 1],
                                                ALU.mult, ALU.add)
                    nc.sync.dma_start(yt_out[:, o * 1024:(o + 1) * 1024],
                                      yt_sb[:, o * 1024:(o + 1) * 1024])

    nc.compile()
    return nc


def _prep_inputs(x, conditioning, w1, b1, w2, b2, bias):
    c = _consts()
    x = np.asarray(x, dtype=np.float32)
    conditioning = np.asarray(conditioning, dtype=np.float32)
    w1 = np.asarray(w1, dtype=np.float32)
    b1 = np.asarray(b1, dtype=np.float32)
    w2 = np.asarray(w2, dtype=np.float32)
    b2 = np.asarray(b2, dtype=np.float32)
    bias = np.asarray(bias, dtype=np.float32)

    w1t = w1.reshape(32, 32, 4, 64).transpose(1, 3, 2, 0).reshape(2048, 4, 32)
    w1n = np.ascontiguousarray(
        w1t.reshape(16, 128, 128).transpose(1, 0, 2).reshape(128, 2048)
    ).astype(np.float16)
    w2t = w2[:, :, 0].T                                               # [32, 64]
    w2r = np.concatenate([np.tile(w2t[:, 0:32], (1, 4)),
                          np.tile(w2t[:, 32:64], (1, 4))],
                         axis=1).astype(np.float16)                   # [32, 256]
    bias64 = np.tile(bias.reshape(1, 4), (64, 1)).astype(np.float32)
    blk = np.arange(128) // 32
    s1 = (blk % 2 == 1).astype(np.float32)
    s2 = (blk >= 2).astype(np.float32)
    mk = np.stack([s1, 1.0 - s1, s2, 1.0 - s2], axis=1)               # [128, 4]

    shared = {
        "eye_in": np.eye(128, dtype=np.float16),
        "w1n_in": w1n, "w2r_in": w2r, "ws_in": c["WS"],
        "wx_in": c["wx_h"], "ica_in": c["ica"], "icb_in": c["icb"],
        "b1_in": b1.reshape(32, 1).copy(),
        "b1s_in": (0.01 * b1).reshape(32, 1).copy(),
        "b2a_in": np.tile(b2[:32], 4).reshape(128, 1).astype(np.float32),
        "b2w_in": np.tile(b2[32:], 4).reshape(128, 1).astype(np.float32),
        "mk_in": mk, "inv3_in": c["inv3"], "bias_in": bias64,
    }
    in_maps = []
    for b in range(B):
        condpad = np.zeros((CI, T + K), dtype=np.float32)
        condpad[:, 128:128 + T] = conditioning[b]
        d = condpad.reshape(CI, H, 64).transpose(0, 2, 1).reshape(2048, H)
        d = np.ascontiguousarray(d.reshape(16, 128, H)).astype(np.float16)
        xp = np.pad(x[b], ((0, 0), (128, 128)), mode="reflect")
        xd = np.ascontiguousarray(
            xp.reshape(2, H, 64).transpose(0, 2, 1).reshape(2, 64, H)
            .transpose(1, 0, 2).reshape(64, 2 * H)).astype(np.float16)
        m = dict(shared)
        m["d_in"] = d
        m["xd_in"] = xd
        in_maps.append(m)
    return in_maps


def _assemble(results):
    y = np.empty((B, O, T), dtype=np.float32)
    for b in range(B):
        yt = results[b]["yt_out"]                        # [64, 4096]
        y[b] = yt.reshape(64, 4, 1024).transpose(1, 2, 0).reshape(4, T)
    return y


def kernel(x, conditioning, w1, b1, w2, b2, bias):
    from concourse.bass_utils import run_bass_kernel_spmd
    if "nc" not in _prog_cache:
        _prog_cache["nc"] = _build_program()
    nc = _prog_cache["nc"]
    in_maps = _prep_inputs(x, conditioning, w1, b1, w2, b2, bias)
    res = run_bass_kernel_spmd(nc, in_maps, core_ids=list(range(B)))
    return _assemble(res.results)
